# revision 18
# baseline (speedup 1.0000x reference)
"""Distributed attention kernel for one TRN2 chip (8 NeuronCores).

Problem: multi-head cross-attention
  B=4, TQ=512, TKV=4096, D=1024, H=8 heads (head_dim=128)

Sharding (data-parallel x tensor-parallel, per the hint):
  core c in 0..7 -> (batch b = c % 4, head-group g = c // 4)
  Each core computes heads [4g, 4g+4) for its batch: Wq/Wk/Wv column
  shards.  Instead of a tail ReduceScatter of output partials, each
  core AllGathers its per-head normalized U^T tiles with its pair
  partner (c <-> c+4) DURING the attention phase (128KB per head,
  fully overlapped), then computes the full 8-head output projection
  for its half of the output columns locally and DMAs it out.  The
  host concatenates the two column halves per batch.

Device layout (per core; everything transposed so no on-device
transposes are needed - the host passes x^T and mask^T):
  Q^T[dh, t]  = Wq_g^T x_q^T          (kc-outer, 4 psum accumulators,
                per-kc interleaved DMA so MMs start ~2.5us in)
  K^T[dh, T]  = Wk_g^T x_kv^T         (kc-outer per 512-col T-chunk)
  V[T, dh]    = x_kv Wv_g             (kc-outer per T-chunk)
  S^T[T, t]   = K^T_h(block)^T Q^T_h  per head, 32 T-blocks
  P^T         = exp(S^T/sqrt(128)) * mask^T   (no max-subtraction needed:
                scores are O(1) so exp cannot overflow/underflow)
  U^T[dh, t] += V_h(block)^T P^T      accumulated over T-blocks in PSUM
  den[1, t]  += ones^T P^T            (PE ones-matmul = partition sum)
  U^T *= 1/max(den, tiny)             (rows with all-false mask give
                U = 0 exactly, so they stay 0 like the reference wipe)
  ut_h -> pair AllGather -> ut_all[r, h]   (overlapped with head h+1)
  out^T[o_half, t] = sum_{hc in 0..8} Wo[hc, o_half]^T ut_all[hc]
                (hc order puts heads {3,7} last so 24 of 32 MMs overlap
                the final exchange), + bo_half, DMA out.

Matmul inputs are bf16 (PE 4x faster than fp32); PSUM accumulation,
softmax denominators and reciprocal stay fp32.
"""

import sys

if "/opt/trn_rl_repo" not in sys.path:
    sys.path.insert(0, "/opt/trn_rl_repo")

import numpy as np
import ml_dtypes
from contextlib import ExitStack

B, TQ, TKV, D, H = 4, 512, 4096, 1024, 8
HD = D // H            # 128 head dim
NCORES = 8
GH = H // 2            # heads per core = 4
GD = GH * HD           # 512 cols per head-group
P = 128
KC = D // P            # 8 contraction chunks
NTB = TKV // P         # 32 T-blocks
NTC = TKV // 512       # 8 T-chunks (DMA granularity)
OHALF = D // 2         # 512 output cols per core
NOB = OHALF // P       # 4 output blocks per core
SCALE = float(1.0 / np.sqrt(HD))

_CACHED_NC = None


def _build_nc():
    from concourse import mybir, bacc
    from concourse.tile import TileContext

    bf = mybir.dt.bfloat16
    f32 = mybir.dt.float32
    AF = mybir.ActivationFunctionType
    OP = mybir.AluOpType

    nc = bacc.Bacc("TRN2", target_bir_lowering=False, debug=False,
                   num_devices=NCORES)

    # All inputs are pre-tiled on the host into partition-major layouts
    # so every DMA is 128 contiguous multi-KB descriptors.
    xqT = nc.dram_tensor("xqT", [P, KC, TQ], bf, kind="ExternalInput")
    xkvT = nc.dram_tensor("xkvT", [P, NTC, KC, 512], bf, kind="ExternalInput")
    maskT = nc.dram_tensor("maskT", [P, NTB, TQ], bf, kind="ExternalInput")
    Wq = nc.dram_tensor("Wq", [P, KC, GD], bf, kind="ExternalInput")
    Wk = nc.dram_tensor("Wk", [P, KC, GD], bf, kind="ExternalInput")
    Wv = nc.dram_tensor("Wv", [P, KC, GD], bf, kind="ExternalInput")
    Wo = nc.dram_tensor("Wo", [P, H, OHALF], bf, kind="ExternalInput")
    bq = nc.dram_tensor("bq", [GD], f32, kind="ExternalInput")
    bk = nc.dram_tensor("bk", [GD], f32, kind="ExternalInput")
    bv = nc.dram_tensor("bv", [GD], f32, kind="ExternalInput")
    bo = nc.dram_tensor("bo", [OHALF], f32, kind="ExternalInput")
    out = nc.dram_tensor("out", [P, NOB, TQ], bf, kind="ExternalOutput")

    with TileContext(nc) as tc:
        with ExitStack() as ctx:
            persist = ctx.enter_context(tc.tile_pool(name="persist", bufs=1))
            kvchunk = ctx.enter_context(tc.tile_pool(name="kvchunk", bufs=2))
            work = ctx.enter_context(tc.tile_pool(name="work", bufs=3))
            outp = ctx.enter_context(tc.tile_pool(name="outp", bufs=1))
            # One pool of double-bank [P, 2, TQ] psum tiles serves the
            # projections (pairs of tiles = 4 accumulators for kc-outer
            # loops), the attention S-tiles (both halves -> one wide exp
            # per pair of T-blocks), and the out-proj accumulators.
            ppool = ctx.enter_context(
                tc.tile_pool(name="ppool", bufs=2, space="PSUM"))
            upool = ctx.enter_context(
                tc.tile_pool(name="upool", bufs=2, space="PSUM"))
            dpool = ctx.enter_context(
                tc.tile_pool(name="dpool", bufs=2, space="PSUM"))
            dram = ctx.enter_context(
                tc.tile_pool(name="dram", bufs=1, space="DRAM"))

            # ---- constants / weights / biases -------------------------
            # DMA emission order matters for time-to-first-matmul: wq/xq
            # interleaved per-kc so the Q projection (kc-outer) starts as
            # soon as the first 256KB lands; then wk+kv0 per-kc, wv, kv1.
            # mask/Wo are only needed later.
            wq_sb = persist.tile([P, KC, GD], bf)
            xq_sb = persist.tile([P, KC, TQ], bf)
            wk_sb = persist.tile([P, KC, GD], bf)
            wv_sb = persist.tile([P, KC, GD], bf)
            # weights on the sync DMA queue, activations on the scalar
            # queue (idle until the first exp ~170us in): parallel
            # descriptor generation halves time-to-kc-slice at startup.
            for kp in range(0, KC, 2):
                nc.sync.dma_start(wq_sb[:, kp:kp + 2, :], Wq.ap()[:, kp:kp + 2, :])
                nc.scalar.dma_start(xq_sb[:, kp:kp + 2, :], xqT.ap()[:, kp:kp + 2, :])

            bq_sb = persist.tile([P, GH], f32)
            bk_sb = persist.tile([P, GH], f32)
            nc.sync.dma_start(bq_sb[:], bq.ap().rearrange("(h p) -> p h", p=P))
            nc.sync.dma_start(bk_sb[:], bk.ap().rearrange("(h p) -> p h", p=P))
            bv_row = persist.tile([1, GD], f32)
            nc.sync.dma_start(bv_row[:], bv.ap().unsqueeze(0))
            bv_rep = persist.tile([P, GD], f32)
            nc.gpsimd.partition_broadcast(bv_rep[:], bv_row[:])

            ones_bf = persist.tile([P, 1], bf)
            nc.vector.memset(ones_bf[:], 1.0)

            kv_tiles = {}

            def load_kv_chunk(tcknk, per_kc=False):
                t = kvchunk.tile([P, KC, 512], bf, name="xkv_t", tag="xkv")
                if per_kc:
                    # chunk 0 feeds the latency-critical first K-proj:
                    # kc-pair slices (weights sync / acts scalar) so each
                    # arrival unblocks the next accumulation steps
                    for kp in range(0, KC, 2):
                        nc.sync.dma_start(wk_sb[:, kp:kp + 2, :],
                                          Wk.ap()[:, kp:kp + 2, :])
                        nc.scalar.dma_start(t[:, kp:kp + 2, :],
                                            xkvT.ap()[:, tcknk, kp:kp + 2, :])
                else:
                    nc.sync.dma_start(t[:], xkvT.ap()[:, tcknk, :, :])
                kv_tiles[tcknk] = t

            load_kv_chunk(0, per_kc=True)
            nc.sync.dma_start(wv_sb[:], Wv.ap())
            load_kv_chunk(1)

            def acc2():
                """One [P,2,TQ] psum tile = 2 independent accumulators.
                Working in pairs staggers the psum recycle: each tile's
                bias-adds hide under the next pair's 16 matmuls."""
                t = ppool.tile([P, 2, TQ], f32, name="proj_ps", tag="big")
                return [t[:, 0, :], t[:, 1, :]]

            # ---- Q^T = Wq_g^T x_q^T  (+bq), kc-outer in db-pairs ------
            qt_sb = persist.tile([P, GH, TQ], bf)
            for dp in range(2):
                qacc = acc2()
                for kc in range(KC):
                    for i in range(2):
                        db = dp * 2 + i
                        nc.tensor.matmul(qacc[i],
                                         wq_sb[:, kc, db * P:(db + 1) * P],
                                         xq_sb[:, kc, :],
                                         start=(kc == 0), stop=(kc == KC - 1))
                for i in range(2):
                    db = dp * 2 + i
                    nc.vector.tensor_tensor(
                        qt_sb[:, db, :], qacc[i],
                        bq_sb[:, db:db + 1].to_broadcast([P, TQ]), OP.add)

            # ---- K^T and V over T-chunks, kc-outer --------------------
            kt_sb = persist.tile([P, GH, TKV], bf)
            v_sb = persist.tile([P, NTB, GD], bf)
            mask_sb = persist.tile([P, NTB, TQ], bf)
            bo_sb = persist.tile([P, NOB], f32)
            wo_sb = persist.tile([P, H, OHALF], bf)
            for tcknk in range(NTC):
                if 1 <= tcknk < NTC - 1:
                    load_kv_chunk(tcknk + 1)
                xkv_t = kv_tiles.pop(tcknk)
                if tcknk == 1:
                    # queue the bulk "later-phase" loads behind chunks 0-1
                    nc.sync.dma_start(mask_sb[:], maskT.ap())
                    nc.sync.dma_start(wo_sb[:], Wo.ap())
                    nc.sync.dma_start(
                        bo_sb[:], bo.ap().rearrange("(ob p) -> p ob", p=P))
                for dp in range(2):
                    kacc = acc2()
                    for kc in range(KC):
                        for i in range(2):
                            db = dp * 2 + i
                            nc.tensor.matmul(kacc[i],
                                             wk_sb[:, kc, db * P:(db + 1) * P],
                                             xkv_t[:, kc, :],
                                             start=(kc == 0),
                                             stop=(kc == KC - 1))
                    for i in range(2):
                        db = dp * 2 + i
                        nc.vector.tensor_tensor(
                            kt_sb[:, db, tcknk * 512:(tcknk + 1) * 512],
                            kacc[i],
                            bk_sb[:, db:db + 1].to_broadcast([P, 512]), OP.add)
                for tp in range(2):
                    vacc = acc2()
                    for kc in range(KC):
                        for i in range(2):
                            tb = tp * 2 + i
                            nc.tensor.matmul(vacc[i],
                                             xkv_t[:, kc, tb * P:(tb + 1) * P],
                                             wv_sb[:, kc, :],
                                             start=(kc == 0),
                                             stop=(kc == KC - 1))
                    for i in range(2):
                        tb = tp * 2 + i
                        nc.vector.tensor_tensor(
                            v_sb[:, tcknk * 4 + tb, :], vacc[i], bv_rep[:],
                            OP.add)

            # ---- attention, flattened double-step loop ----------------
            # Two T-blocks per step: two S-matmuls fill the two banks of
            # one [P, 2, TQ] psum tile, then ONE wide exp (ACT per-op
            # overhead amortized below the PE pace) and one wide mask-mult.
            ut_sb = persist.tile([P, GH, TQ], bf)
            # one receive tile per head: separate tiles keep the write
            # regions trivially disjoint so out-proj MMs for head h only
            # dep on head h's receive (a single shared tile with 8 slice
            # writers serialized the whole out-proj behind late receives)
            ut_t = [persist.tile([P, 2, TQ], bf, name=f"ut_t{h}")
                    for h in range(GH)]
            # heads {0,1} share one AllGather (triggered at head 1's end)
            # so only 3 ops hit the CC ring and it is idle again by the
            # time head 3's (latency-critical) exchange triggers.
            ag_in01 = dram.tile([2, P, TQ], bf, name="ag_in01")
            ag_out01 = dram.tile([2, 2, P, TQ], bf, name="ag_out01")
            ag_in = {h: dram.tile([P, TQ], bf, name=f"ag_in{h}")
                     for h in (2, 3)}
            ag_out = {h: dram.tile([2, P, TQ], bf, name=f"ag_out{h}")
                      for h in (2, 3)}
            NDS = GH * NTB // 2
            s_tiles = {}
            u_tiles = [None] * GH
            den_tiles = [None] * GH
            SPRE = 2  # double-step prefetch depth

            def s2_mm(ds):
                t2 = ppool.tile([P, 2, TQ], f32, name="s2_ps", tag="big")
                for k in range(2):
                    h, j = divmod(ds * 2 + k, NTB)
                    nc.tensor.matmul(t2[:, k, :],
                                     kt_sb[:, h, j * P:(j + 1) * P],
                                     qt_sb[:, h, :], start=True, stop=True)
                return t2

            for pre in range(SPRE):
                s_tiles[pre] = s2_mm(pre)
            for ds in range(NDS):
                h, j0 = divmod(ds * 2, NTB)
                if j0 == 0:
                    u_tiles[h] = upool.tile([P, TQ], f32, name="u_ps",
                                            tag="u_ps")
                    den_tiles[h] = dpool.tile([1, TQ], f32, name="den_ps",
                                              tag="den_ps")
                t2 = s_tiles.pop(ds)
                praw = work.tile([P, 2, TQ], bf, tag="praw", bufs=2)
                nc.scalar.activation(praw[:], t2[:], AF.Exp, scale=SCALE)
                p_t = work.tile([P, 2, TQ], bf, tag="p_t", bufs=2)
                nc.vector.tensor_tensor(p_t[:], praw[:],
                                        mask_sb[:, j0:j0 + 2, :], OP.mult)
                # pair-sum on DVE (cheap bf16 2x op) halves the den
                # matmul count: den += ones^T (p0 + p1)
                p_sum = work.tile([P, TQ], bf, tag="p_sum", bufs=2)
                nc.vector.tensor_tensor(p_sum[:], p_t[:, 0, :], p_t[:, 1, :],
                                        OP.add)
                if ds + SPRE < NDS:
                    s_tiles[ds + SPRE] = s2_mm(ds + SPRE)
                for k in range(2):
                    j = j0 + k
                    nc.tensor.matmul(u_tiles[h][:],
                                     v_sb[:, j, h * P:(h + 1) * P],
                                     p_t[:, k, :],
                                     start=(j == 0), stop=(j == NTB - 1))
                nc.tensor.matmul(den_tiles[h][:], ones_bf[:], p_sum[:],
                                 start=(j0 == 0), stop=(j0 + 2 == NTB))
                if j0 + 2 == NTB:
                    # head h done: normalize U^T and kick off the pair
                    # exchange (fully overlapped with head h+1's MMs).
                    # max(den, tiny) as ACT copy-with-bias: den >= 0 so
                    # den + 1e-30 == max(den, 1e-30) to within fp32 eps.
                    den_sf = work.tile([1, TQ], f32, tag="den_sf")
                    nc.scalar.activation(den_sf[:], den_tiles[h][:], AF.Copy,
                                         bias=1e-30)
                    recip = work.tile([1, TQ], f32, tag="recip")
                    nc.vector.reciprocal_approx_fast(recip[:], den_sf[:])
                    recip_rep = work.tile([P, TQ], f32, tag="recip_rep")
                    nc.gpsimd.partition_broadcast(recip_rep[:], recip[:])
                    nc.vector.tensor_tensor(ut_sb[:, h, :], u_tiles[h][:],
                                            recip_rep[:], OP.mult)
                    # send on the gpsimd queue (it triggers the collective
                    # right after anyway); receives on sync.  Keeping them
                    # on separate queues stops a receive's wait-for-AG_h
                    # from blocking the next head's send in the FIFO.
                    groups = [[0, 4], [1, 5], [2, 6], [3, 7]]
                    if h < 2:
                        nc.gpsimd.dma_start(ag_in01[h, :, :], ut_sb[:, h, :])
                        if h == 1:
                            nc.gpsimd.collective_compute(
                                "AllGather", mybir.AluOpType.bypass,
                                replica_groups=groups,
                                ins=[ag_in01.opt()], outs=[ag_out01.opt()],
                            )
                            for hh in range(2):
                                for r in range(2):
                                    nc.sync.dma_start(
                                        ut_t[hh][:, r, :],
                                        ag_out01[r, hh, :, :])
                    else:
                        nc.gpsimd.dma_start(ag_in[h][:], ut_sb[:, h, :])
                        nc.gpsimd.collective_compute(
                            "AllGather", mybir.AluOpType.bypass,
                            replica_groups=groups,
                            ins=[ag_in[h].opt()], outs=[ag_out[h].opt()],
                        )
                        for r in range(2):
                            nc.sync.dma_start(ut_t[h][:, r, :],
                                              ag_out[h][r, :, :])

            # ---- out^T[o_half] = sum_hc Wo[hc]^T ut_all[hc] (+bo) -----
            # hc-outer with heads {3,7} last: the 24 MMs for heads
            # 0-2/4-6 run while head 3's exchange is still in flight.
            o_out = outp.tile([P, NOB, TQ], bf, name="o_out", tag="o_out")
            oacc = acc2() + acc2()
            order = [0, 1, 2, 4, 5, 6, 3, 7]
            for idx, hc in enumerate(order):
                if idx == 6:
                    # heads {3,7} wait on the final AllGather (~5us): keep
                    # the PE busy with throwaway [1,512] matmuls so HAM
                    # doesn't re-throttle to half clock for the last MMs
                    warm = upool.tile([P, TQ], f32, name="u_ps", tag="u_ps")
                    for _ in range(20):
                        nc.tensor.matmul(warm[0:1, :], ones_bf[:],
                                         qt_sb[:, 0, :], start=True, stop=True)
                r, hl = divmod(hc, GH)
                for ob in range(NOB):
                    nc.tensor.matmul(oacc[ob],
                                     wo_sb[:, hc, ob * P:(ob + 1) * P],
                                     ut_t[hl][:, r, :],
                                     start=(idx == 0), stop=(idx == H - 1))
            for ob in range(NOB):
                nc.vector.tensor_tensor(
                    o_out[:, ob, :], oacc[ob],
                    bo_sb[:, ob:ob + 1].to_broadcast([P, TQ]), OP.add)
                nc.sync.dma_start(out.ap()[:, ob:ob + 1, :],
                                  o_out[:, ob:ob + 1, :])

    nc.finalize()
    return nc


def _shard_inputs(inputs_q, inputs_kv, attention_mask, Wq, bq, Wk, bk, Wv, bv,
                  Wo, bo):
    bf16 = ml_dtypes.bfloat16
    f32 = np.float32

    def ptile(a2d, inner):
        """[R, C] row-major -> [P, R//P, C] partition-major, contiguous."""
        r, c = a2d.shape
        return np.ascontiguousarray(
            a2d.reshape(r // P, P, c).transpose(1, 0, 2)).astype(inner)

    in_maps = []
    xqT = [ptile(inputs_q[b].T, bf16) for b in range(B)]          # [P,KC,TQ]
    xkvT = [ptile(inputs_kv[b].T, bf16)                           # [P,NTC,KC,512]
            .reshape(P, KC, NTC, 512).transpose(0, 2, 1, 3).copy()
            for b in range(B)]
    maskT = [ptile(attention_mask[b].T.astype(np.float32), bf16)  # [P,NTB,TQ]
             for b in range(B)]
    for c in range(NCORES):
        b, g = c % B, c // B  # pair = (b, b+4)
        sl = slice(g * GD, (g + 1) * GD)
        osl = slice(g * OHALF, (g + 1) * OHALF)
        in_maps.append({
            "xqT": xqT[b],
            "xkvT": xkvT[b],
            "maskT": maskT[b],
            "Wq": ptile(np.ascontiguousarray(Wq[:, sl]), bf16),
            "Wk": ptile(np.ascontiguousarray(Wk[:, sl]), bf16),
            "Wv": ptile(np.ascontiguousarray(Wv[:, sl]), bf16),
            "Wo": ptile(np.ascontiguousarray(Wo[:, osl]), bf16),
            "bq": np.ascontiguousarray(bq[sl]).astype(f32),
            "bk": np.ascontiguousarray(bk[sl]).astype(f32),
            "bv": np.ascontiguousarray(bv[sl]).astype(f32),
            "bo": np.ascontiguousarray(bo[osl]).astype(f32),
        })
    return in_maps


def kernel(_trace=False, **inputs):
    global _CACHED_NC
    from concourse import bass_utils

    arrs = {k: np.asarray(v) for k, v in inputs.items()}
    in_maps = _shard_inputs(**arrs)

    if _CACHED_NC is None:
        _CACHED_NC = _build_nc()

    res = bass_utils.run_bass_kernel_spmd(
        _CACHED_NC, in_maps, core_ids=list(range(NCORES)), trace=_trace)

    full = np.empty((B, TQ, D), np.float32)
    for b in range(B):
        # core b holds output cols [0, 512), core b+4 holds [512, 1024)
        for c in (b, b + 4):
            g = c // B
            arr = res.results[c]["out"]  # [P, NOB, TQ] bf16, o = ob*128+p
            outT_half = arr.transpose(1, 0, 2).reshape(OHALF, TQ)
            full[b][:, g * OHALF:(g + 1) * OHALF] = outT_half.T
    if _trace:
        return full, res
    return full


# revision 19
# speedup vs baseline: 1.2211x; 1.2211x over previous
"""Distributed attention kernel for one TRN2 chip (8 NeuronCores).

Problem: multi-head cross-attention
  B=4, TQ=512, TKV=4096, D=1024, H=8 heads (head_dim=128)

Sharding (data-parallel x tensor-parallel, per the hint):
  core c in 0..7 -> (batch b = c % 4, head-group g = c // 4)
  Each core computes heads [4g, 4g+4) for its batch: Wq/Wk/Wv column
  shards.  Instead of a tail ReduceScatter of output partials, each
  core AllGathers its per-head normalized U^T tiles with its pair
  partner (c <-> c+4) DURING the attention phase (128KB per head,
  fully overlapped), then computes the full 8-head output projection
  for its half of the output columns locally and DMAs it out.  The
  host concatenates the two column halves per batch.

Device schedule (per core):
  Q^T = Wq_g^T x_q^T               kc-outer in db-pairs, per-kc-pair DMA
  per 512-col T-chunk t:           K^T then V projections (kc-outer in
                                   pairs), with HEAD 0's attention for
                                   chunk t-1's T-blocks interleaved
                                   between the K pair-groups -- this
                                   spreads head 0's exp/mask load into
                                   the projection phase, shortening the
                                   ACT/DVE-bound attention phase by 1/4
  heads 1..3:                      flattened double-step loop
     S^T = K^T(block)^T Q^T        two T-blocks fill one [P,2,TQ] psum
     P^T = exp(S^T/sqrt(dh))*mask  one wide exp (ACT), one wide mult
     U^T += V(block)^T P^T         PSUM accumulation
     den += ones^T (P0+P1)         DVE pair-add halves den matmuls; the
                                   pair roundings are independent so the
                                   4096-term denominator sum averages
                                   them out (<0.01% den error)
  per-head tail: U psum -> SBUF via ACT copy (frees the single U psum
     bank early), den+1e-30 (ACT), reciprocal_approx_fast (DVE),
     partition_broadcast (gpsimd), normalize, then pair AllGather
     (heads {0,1} batched into one ring op; 2, 3 individual) with sends
     on the gpsimd queue and receives on sync so nothing serializes.
  out-proj: hc-outer, heads {3,7} last behind warm-keeper matmuls.

Matmul inputs are bf16 (PE 4x faster than fp32); PSUM accumulation,
softmax denominators and reciprocal stay fp32.  No max-subtraction in
softmax: scores are O(1) so exp cannot overflow/underflow.  Rows whose
mask is all-false give U = 0 exactly, matching the reference wipe.
"""

import sys

if "/opt/trn_rl_repo" not in sys.path:
    sys.path.insert(0, "/opt/trn_rl_repo")

import numpy as np
import ml_dtypes
from contextlib import ExitStack

B, TQ, TKV, D, H = 4, 512, 4096, 1024, 8
HD = D // H            # 128 head dim
NCORES = 8
GH = H // 2            # heads per core = 4
GD = GH * HD           # 512 cols per head-group
P = 128
KC = D // P            # 8 contraction chunks
NTB = TKV // P         # 32 T-blocks
NTC = TKV // 512       # 8 T-chunks (DMA granularity)
OHALF = D // 2         # 512 output cols per core
NOB = OHALF // P       # 4 output blocks per core
SCALE = float(1.0 / np.sqrt(HD))

_CACHED_NC = None


def _build_nc():
    from concourse import mybir, bacc
    from concourse.tile import TileContext

    bf = mybir.dt.bfloat16
    f32 = mybir.dt.float32
    AF = mybir.ActivationFunctionType
    OP = mybir.AluOpType

    nc = bacc.Bacc("TRN2", target_bir_lowering=False, debug=False,
                   num_devices=NCORES)

    # All inputs are pre-tiled on the host into partition-major layouts
    # so every DMA is 128 contiguous multi-KB descriptors.
    xqT = nc.dram_tensor("xqT", [P, KC, TQ], bf, kind="ExternalInput")
    xkvT = nc.dram_tensor("xkvT", [P, NTC, KC, 512], bf, kind="ExternalInput")
    maskT = nc.dram_tensor("maskT", [P, NTB, TQ], bf, kind="ExternalInput")
    Wq = nc.dram_tensor("Wq", [P, KC, GD], bf, kind="ExternalInput")
    Wk = nc.dram_tensor("Wk", [P, KC, GD], bf, kind="ExternalInput")
    Wv = nc.dram_tensor("Wv", [P, KC, GD], bf, kind="ExternalInput")
    Wo = nc.dram_tensor("Wo", [P, H, OHALF], bf, kind="ExternalInput")
    bq = nc.dram_tensor("bq", [GD], f32, kind="ExternalInput")
    bk = nc.dram_tensor("bk", [GD], f32, kind="ExternalInput")
    bv = nc.dram_tensor("bv", [GD], f32, kind="ExternalInput")
    bo = nc.dram_tensor("bo", [OHALF], f32, kind="ExternalInput")
    out = nc.dram_tensor("out", [P, NOB, TQ], bf, kind="ExternalOutput")

    with TileContext(nc) as tc:
        with ExitStack() as ctx:
            persist = ctx.enter_context(tc.tile_pool(name="persist", bufs=1))
            kvchunk = ctx.enter_context(tc.tile_pool(name="kvchunk", bufs=2))
            work = ctx.enter_context(tc.tile_pool(name="work", bufs=3))
            outp = ctx.enter_context(tc.tile_pool(name="outp", bufs=1))
            # PSUM budget (8 banks): ppool 3x[P,2,TQ] = 6 banks (proj
            # accumulator pairs, S-tiles, out-proj accumulators), upool
            # 1 bank (one U accumulator at a time; the ACT copy at each
            # head's end frees it before the next head needs it), dpool
            # 1 bank (den, freed by the ACT den_sf copy).
            ppool = ctx.enter_context(
                tc.tile_pool(name="ppool", bufs=3, space="PSUM"))
            upool = ctx.enter_context(
                tc.tile_pool(name="upool", bufs=1, space="PSUM"))
            dpool = ctx.enter_context(
                tc.tile_pool(name="dpool", bufs=1, space="PSUM"))
            dram = ctx.enter_context(
                tc.tile_pool(name="dram", bufs=1, space="DRAM"))

            # ---- constants / weights / biases -------------------------
            # weights on the sync DMA queue, activations on the scalar
            # queue (idle until the first exp): parallel descriptor
            # generation halves time-to-kc-slice at startup.
            wq_sb = persist.tile([P, KC, GD], bf)
            xq_sb = persist.tile([P, KC, TQ], bf)
            wk_sb = persist.tile([P, KC, GD], bf)
            wv_sb = persist.tile([P, KC, GD], bf)
            for kp in range(0, KC, 2):
                nc.sync.dma_start(wq_sb[:, kp:kp + 2, :], Wq.ap()[:, kp:kp + 2, :])
                nc.scalar.dma_start(xq_sb[:, kp:kp + 2, :], xqT.ap()[:, kp:kp + 2, :])

            bq_sb = persist.tile([P, GH], f32)
            bk_sb = persist.tile([P, GH], f32)
            nc.sync.dma_start(bq_sb[:], bq.ap().rearrange("(h p) -> p h", p=P))
            nc.sync.dma_start(bk_sb[:], bk.ap().rearrange("(h p) -> p h", p=P))
            bv_row = persist.tile([1, GD], f32)
            nc.sync.dma_start(bv_row[:], bv.ap().unsqueeze(0))
            bv_rep = persist.tile([P, GD], f32)
            nc.gpsimd.partition_broadcast(bv_rep[:], bv_row[:])

            ones_bf = persist.tile([P, 1], bf)
            nc.vector.memset(ones_bf[:], 1.0)

            mask_sb = persist.tile([P, NTB, TQ], bf)
            kv_tiles = {}

            def load_kv_chunk(tcknk, per_kc=False):
                t = kvchunk.tile([P, KC, 512], bf, name="xkv_t", tag="xkv")
                if per_kc:
                    # chunk 0 feeds the latency-critical first K-proj:
                    # kc-pair slices (weights sync / acts scalar) so each
                    # arrival unblocks the next accumulation steps
                    for kp in range(0, KC, 2):
                        nc.sync.dma_start(wk_sb[:, kp:kp + 2, :],
                                          Wk.ap()[:, kp:kp + 2, :])
                        nc.scalar.dma_start(t[:, kp:kp + 2, :],
                                            xkvT.ap()[:, tcknk, kp:kp + 2, :])
                else:
                    nc.sync.dma_start(t[:], xkvT.ap()[:, tcknk, :, :])
                # head 0 consumes chunk t's mask blocks one chunk later
                nc.sync.dma_start(mask_sb[:, tcknk * 4:tcknk * 4 + 4, :],
                                  maskT.ap()[:, tcknk * 4:tcknk * 4 + 4, :])
                kv_tiles[tcknk] = t

            load_kv_chunk(0, per_kc=True)
            nc.sync.dma_start(wv_sb[:], Wv.ap())
            load_kv_chunk(1)

            def acc2():
                """One [P,2,TQ] psum tile = 2 independent accumulators.
                Working in pairs staggers the psum recycle: each tile's
                bias-adds hide under the next pair's 16 matmuls."""
                t = ppool.tile([P, 2, TQ], f32, name="proj_ps", tag="big")
                return [t[:, 0, :], t[:, 1, :]]

            # ---- Q^T = Wq_g^T x_q^T  (+bq), kc-outer in db-pairs ------
            qt_sb = persist.tile([P, GH, TQ], bf)
            for dp in range(2):
                qacc = acc2()
                for kc in range(KC):
                    for i in range(2):
                        db = dp * 2 + i
                        nc.tensor.matmul(qacc[i],
                                         wq_sb[:, kc, db * P:(db + 1) * P],
                                         xq_sb[:, kc, :],
                                         start=(kc == 0), stop=(kc == KC - 1))
                for i in range(2):
                    db = dp * 2 + i
                    nc.vector.tensor_tensor(
                        qt_sb[:, db, :], qacc[i],
                        bq_sb[:, db:db + 1].to_broadcast([P, TQ]), OP.add)

            # ---- attention helpers ------------------------------------
            kt_sb = persist.tile([P, GH, TKV], bf)
            v_sb = persist.tile([P, NTB, GD], bf)
            bo_sb = persist.tile([P, NOB], f32)
            wo_sb = persist.tile([P, H, OHALF], bf)
            ut_sb = persist.tile([P, GH, TQ], bf)
            # one receive tile per head: separate tiles keep the write
            # regions trivially disjoint so out-proj MMs for head h only
            # dep on head h's receive
            ut_t = [persist.tile([P, 2, TQ], bf, name=f"ut_t{h}")
                    for h in range(GH)]
            # heads {0,1} share one AllGather (triggered at head 1's end)
            # so only 3 ops hit the CC ring and it is idle again by the
            # time head 3's (latency-critical) exchange triggers.
            ag_in01 = dram.tile([2, P, TQ], bf, name="ag_in01")
            ag_out01 = dram.tile([2, 2, P, TQ], bf, name="ag_out01")
            ag_in = {h: dram.tile([P, TQ], bf, name=f"ag_in{h}")
                     for h in (2, 3)}
            ag_out = {h: dram.tile([2, P, TQ], bf, name=f"ag_out{h}")
                      for h in (2, 3)}
            cur = {}  # current head's u/den psum tiles

            def s2_mm(h, ds):
                t2 = ppool.tile([P, 2, TQ], f32, name="s2_ps", tag="big")
                for k in range(2):
                    j = ds * 2 + k
                    nc.tensor.matmul(t2[:, k, :],
                                     kt_sb[:, h, j * P:(j + 1) * P],
                                     qt_sb[:, h, :], start=True, stop=True)
                return t2

            def att_consume(h, ds, t2):
                j0 = ds * 2
                if j0 == 0:
                    cur["u"] = upool.tile([P, TQ], f32, name="u_ps",
                                          tag="u_ps")
                    cur["den"] = dpool.tile([1, TQ], f32, name="den_ps",
                                            tag="den_ps")
                praw = work.tile([P, 2, TQ], bf, tag="praw", bufs=2)
                nc.scalar.activation(praw[:], t2[:], AF.Exp, scale=SCALE)
                p_t = work.tile([P, 2, TQ], bf, tag="p_t", bufs=2)
                nc.vector.tensor_tensor(p_t[:], praw[:],
                                        mask_sb[:, j0:j0 + 2, :], OP.mult)
                # pair-sum on DVE (cheap bf16 2x op) halves the den
                # matmul count: den += ones^T (p0 + p1)
                p_sum = work.tile([P, TQ], bf, tag="p_sum", bufs=2)
                nc.vector.tensor_tensor(p_sum[:], p_t[:, 0, :], p_t[:, 1, :],
                                        OP.add)
                for k in range(2):
                    j = j0 + k
                    nc.tensor.matmul(cur["u"][:],
                                     v_sb[:, j, h * P:(h + 1) * P],
                                     p_t[:, k, :],
                                     start=(j == 0), stop=(j == NTB - 1))
                nc.tensor.matmul(cur["den"][:], ones_bf[:], p_sum[:],
                                 start=(j0 == 0), stop=(j0 + 2 == NTB))

            def head_tail(h):
                # ACT copies free the single U/den psum banks early so
                # the next head's accumulation can claim them; the
                # normalize chain then runs entirely from SBUF.
                u_sf = work.tile([P, TQ], f32, tag="u_sf", bufs=2)
                nc.scalar.activation(u_sf[:], cur["u"][:], AF.Copy)
                # den >= 0 so den + 1e-30 == max(den, 1e-30) in fp32
                den_sf = work.tile([1, TQ], f32, tag="den_sf")
                nc.scalar.activation(den_sf[:], cur["den"][:], AF.Copy,
                                     bias=1e-30)
                recip = work.tile([1, TQ], f32, tag="recip")
                nc.vector.reciprocal_approx_fast(recip[:], den_sf[:])
                recip_rep = work.tile([P, TQ], f32, tag="recip_rep")
                nc.gpsimd.partition_broadcast(recip_rep[:], recip[:])
                nc.vector.tensor_tensor(ut_sb[:, h, :], u_sf[:],
                                        recip_rep[:], OP.mult)
                # send on the gpsimd queue (it triggers the collective
                # right after anyway); receives on sync.  Separate queues
                # stop a receive's wait-for-AG from blocking the next
                # head's send in the FIFO.
                groups = [[0, 4], [1, 5], [2, 6], [3, 7]]
                if h < 2:
                    nc.gpsimd.dma_start(ag_in01[h, :, :], ut_sb[:, h, :])
                    if h == 1:
                        nc.gpsimd.collective_compute(
                            "AllGather", mybir.AluOpType.bypass,
                            replica_groups=groups,
                            ins=[ag_in01.opt()], outs=[ag_out01.opt()],
                        )
                        for hh in range(2):
                            for r in range(2):
                                nc.sync.dma_start(ut_t[hh][:, r, :],
                                                  ag_out01[r, hh, :, :])
                else:
                    nc.gpsimd.dma_start(ag_in[h][:], ut_sb[:, h, :])
                    nc.gpsimd.collective_compute(
                        "AllGather", mybir.AluOpType.bypass,
                        replica_groups=groups,
                        ins=[ag_in[h].opt()], outs=[ag_out[h].opt()],
                    )
                    for r in range(2):
                        nc.sync.dma_start(ut_t[h][:, r, :],
                                          ag_out[h][r, :, :])

            # ---- K^T and V over T-chunks, head 0 interleaved ----------
            # Chunk t's K/V projections surround head 0's attention for
            # chunk t-1's T-blocks: the S matmuls sit between the two
            # K pair-groups so their exp/mask chains complete under the
            # following 16 projection MMs and the U matmuls never stall.
            for tcknk in range(NTC):
                if 1 <= tcknk < NTC - 1:
                    load_kv_chunk(tcknk + 1)
                xkv_t = kv_tiles.pop(tcknk)
                if tcknk == 1:
                    nc.sync.dma_start(wo_sb[:], Wo.ap())
                    nc.sync.dma_start(
                        bo_sb[:], bo.ap().rearrange("(ob p) -> p ob", p=P))
                s_pair = None
                for dp in range(2):
                    kacc = acc2()
                    for kc in range(KC):
                        for i in range(2):
                            db = dp * 2 + i
                            nc.tensor.matmul(kacc[i],
                                             wk_sb[:, kc, db * P:(db + 1) * P],
                                             xkv_t[:, kc, :],
                                             start=(kc == 0),
                                             stop=(kc == KC - 1))
                    for i in range(2):
                        db = dp * 2 + i
                        nc.vector.tensor_tensor(
                            kt_sb[:, db, tcknk * 512:(tcknk + 1) * 512],
                            kacc[i],
                            bk_sb[:, db:db + 1].to_broadcast([P, 512]), OP.add)
                    if tcknk >= 1:
                        if dp == 0:
                            ds0 = 2 * (tcknk - 1)
                            s_pair = [s2_mm(0, ds0), s2_mm(0, ds0 + 1)]
                        else:
                            ds0 = 2 * (tcknk - 1)
                            att_consume(0, ds0, s_pair[0])
                            att_consume(0, ds0 + 1, s_pair[1])
                for tp in range(2):
                    vacc = acc2()
                    for kc in range(KC):
                        for i in range(2):
                            tb = tp * 2 + i
                            nc.tensor.matmul(vacc[i],
                                             xkv_t[:, kc, tb * P:(tb + 1) * P],
                                             wv_sb[:, kc, :],
                                             start=(kc == 0),
                                             stop=(kc == KC - 1))
                    for i in range(2):
                        tb = tp * 2 + i
                        nc.vector.tensor_tensor(
                            v_sb[:, tcknk * 4 + tb, :], vacc[i], bv_rep[:],
                            OP.add)
            # head 0's last chunk of T-blocks + its tail
            s_pair = [s2_mm(0, 2 * (NTC - 1)), s2_mm(0, 2 * (NTC - 1) + 1)]
            att_consume(0, 2 * (NTC - 1), s_pair[0])
            att_consume(0, 2 * (NTC - 1) + 1, s_pair[1])
            head_tail(0)

            # ---- heads 1..3: flattened double-step loop ---------------
            HDS = NTB // 2  # 16 double-steps per head
            NDS = 3 * HDS
            s_tiles = {}
            SPRE = 2

            def hd(ds):
                return 1 + ds // HDS, ds % HDS

            for pre in range(SPRE):
                s_tiles[pre] = s2_mm(*hd(pre))
            for ds in range(NDS):
                h, dsl = hd(ds)
                t2 = s_tiles.pop(ds)
                if ds + SPRE < NDS:
                    s_tiles[ds + SPRE] = s2_mm(*hd(ds + SPRE))
                att_consume(h, dsl, t2)
                if dsl == HDS - 1:
                    head_tail(h)

            # ---- out^T[o_half] = sum_hc Wo[hc]^T ut[hc] (+bo) ---------
            # hc-outer with heads {3,7} last: the 24 MMs for heads
            # 0-2/4-6 run while head 3's exchange is still in flight.
            o_out = outp.tile([P, NOB, TQ], bf, name="o_out", tag="o_out")
            oacc = acc2() + acc2()
            order = [0, 1, 2, 4, 5, 6, 3, 7]
            for idx, hc in enumerate(order):
                if idx == 6:
                    # heads {3,7} wait on the final AllGather (~5us): keep
                    # the PE busy with throwaway [1,512] matmuls so HAM
                    # doesn't re-throttle to half clock for the last MMs
                    warm = upool.tile([P, TQ], f32, name="u_ps", tag="u_ps")
                    for _ in range(20):
                        nc.tensor.matmul(warm[0:1, :], ones_bf[:],
                                         qt_sb[:, 0, :], start=True, stop=True)
                r, hl = divmod(hc, GH)
                for ob in range(NOB):
                    nc.tensor.matmul(oacc[ob],
                                     wo_sb[:, hc, ob * P:(ob + 1) * P],
                                     ut_t[hl][:, r, :],
                                     start=(idx == 0), stop=(idx == H - 1))
            for ob in range(NOB):
                nc.vector.tensor_tensor(
                    o_out[:, ob, :], oacc[ob],
                    bo_sb[:, ob:ob + 1].to_broadcast([P, TQ]), OP.add)
                nc.sync.dma_start(out.ap()[:, ob:ob + 1, :],
                                  o_out[:, ob:ob + 1, :])

    nc.finalize()
    return nc


def _shard_inputs(inputs_q, inputs_kv, attention_mask, Wq, bq, Wk, bk, Wv, bv,
                  Wo, bo):
    bf16 = ml_dtypes.bfloat16
    f32 = np.float32

    def ptile(a2d, inner):
        """[R, C] row-major -> [P, R//P, C] partition-major, contiguous."""
        r, c = a2d.shape
        return np.ascontiguousarray(
            a2d.reshape(r // P, P, c).transpose(1, 0, 2)).astype(inner)

    in_maps = []
    xqT = [ptile(inputs_q[b].T, bf16) for b in range(B)]          # [P,KC,TQ]
    xkvT = [ptile(inputs_kv[b].T, bf16)                           # [P,NTC,KC,512]
            .reshape(P, KC, NTC, 512).transpose(0, 2, 1, 3).copy()
            for b in range(B)]
    maskT = [ptile(attention_mask[b].T.astype(np.float32), bf16)  # [P,NTB,TQ]
             for b in range(B)]
    for c in range(NCORES):
        b, g = c % B, c // B  # pair = (b, b+4)
        sl = slice(g * GD, (g + 1) * GD)
        osl = slice(g * OHALF, (g + 1) * OHALF)
        in_maps.append({
            "xqT": xqT[b],
            "xkvT": xkvT[b],
            "maskT": maskT[b],
            "Wq": ptile(np.ascontiguousarray(Wq[:, sl]), bf16),
            "Wk": ptile(np.ascontiguousarray(Wk[:, sl]), bf16),
            "Wv": ptile(np.ascontiguousarray(Wv[:, sl]), bf16),
            "Wo": ptile(np.ascontiguousarray(Wo[:, osl]), bf16),
            "bq": np.ascontiguousarray(bq[sl]).astype(f32),
            "bk": np.ascontiguousarray(bk[sl]).astype(f32),
            "bv": np.ascontiguousarray(bv[sl]).astype(f32),
            "bo": np.ascontiguousarray(bo[osl]).astype(f32),
        })
    return in_maps


def kernel(_trace=False, **inputs):
    global _CACHED_NC
    from concourse import bass_utils

    arrs = {k: np.asarray(v) for k, v in inputs.items()}
    in_maps = _shard_inputs(**arrs)

    if _CACHED_NC is None:
        _CACHED_NC = _build_nc()

    res = bass_utils.run_bass_kernel_spmd(
        _CACHED_NC, in_maps, core_ids=list(range(NCORES)), trace=_trace)

    full = np.empty((B, TQ, D), np.float32)
    for b in range(B):
        # core b holds output cols [0, 512), core b+4 holds [512, 1024)
        for c in (b, b + 4):
            g = c // B
            arr = res.results[c]["out"]  # [P, NOB, TQ] bf16, o = ob*128+p
            outT_half = arr.transpose(1, 0, 2).reshape(OHALF, TQ)
            full[b][:, g * OHALF:(g + 1) * OHALF] = outT_half.T
    if _trace:
        return full, res
    return full
